# revision 6
# baseline (speedup 1.0000x reference)
"""LLRNN Trainium2 kernel.

Reference computation (per batch row b):
    zx[b,t,:] = x[b,t,:] @ K + bias          (K: [64, 256], bias: [256])
    c_t = f * c_{t-1} + (1-f) * tanh(z_i)    where
        z = zx_t + c_{t-1} @ R               (R: [128, 256])
        f = sigmoid(z[:, :128]), z_i = z[:, 128:]
    out[b,:] = c_T @ Wd + bd                 (Wd: [128, 64], bd: [64])

Mapping (per core, batch shard of 512):
  - State kept transposed: c [128 cell partitions, batch free].
  - Host pre-transposes x to [t-pair, 128 (j*64+i), 512 b] fp16 so every DMA
    is a contiguous 128-partition 128KB tile; no on-device transposes.
  - Gate bias is applied for free via the ACT per-partition bias operand.
  - Two independent half-batch chains (A/B, 256 cols each) pipeline across
    PE (matmuls), ACT (sigmoid/tanh), DVE (blend).
  - Final dense layer: c chunks become the matmul stationary so the output
    lands batch-major in PSUM; bias added via DVE with a broadcast tile.
"""

import sys

sys.path.insert(0, "/opt/trn_rl_repo")

import numpy as np

T_FULL = 256
IN_DIMS = 64
CELL = 128
OUT_DIMS = 64
BATCH = 4096
NCORES = 8
BC = BATCH // NCORES  # 512 batch rows per core
HB = BC // 2  # 256: half-batch chain width


def build_nc(T=T_FULL):
    import concourse.bass as bass
    import concourse.tile as tile
    from concourse import bacc, mybir
    from contextlib import ExitStack

    f16 = mybir.dt.float16
    f32 = mybir.dt.float32
    AF = mybir.ActivationFunctionType
    OP = mybir.AluOpType

    nc = bacc.Bacc("TRN2", target_bir_lowering=False, debug=False)

    ntp = T // 2
    xp_d = nc.dram_tensor("xp", [ntp, 128, BC], f16, kind="ExternalInput")
    wk_d = nc.dram_tensor("wk", [128, 2 * CELL], f16, kind="ExternalInput")
    wr_d = nc.dram_tensor("wr", [CELL, 2 * CELL], f16, kind="ExternalInput")
    bf_d = nc.dram_tensor("bf", [CELL, 1], f32, kind="ExternalInput")
    bi_d = nc.dram_tensor("bi", [CELL, 1], f32, kind="ExternalInput")
    dw_d = nc.dram_tensor("dw", [CELL, OUT_DIMS], f16, kind="ExternalInput")
    db_d = nc.dram_tensor("db", [OUT_DIMS], f32, kind="ExternalInput")
    out_d = nc.dram_tensor("out", [BC, OUT_DIMS], f32, kind="ExternalOutput")

    with tile.TileContext(nc) as tc, ExitStack() as ctx:
        wpool = ctx.enter_context(tc.tile_pool(name="w", bufs=1))
        xpool = ctx.enter_context(tc.tile_pool(name="x", bufs=8))
        gpool = ctx.enter_context(tc.tile_pool(name="g", bufs=3))
        spool = ctx.enter_context(tc.tile_pool(name="s", bufs=1))
        zpool = ctx.enter_context(tc.tile_pool(name="z", bufs=2, space="PSUM"))

        # ---- constants / weights ----
        # wk stacked twice so odd-timestep rhs slices (partition base 64)
        # pair with a weight slice at the same base partition
        wk_sb = wpool.tile([128, 2 * CELL], f16, tag="wk")
        nc.sync.dma_start(wk_sb[:], wk_d[:])
        wr_sb = wpool.tile([CELL, 2 * CELL], f16, tag="wr")
        nc.sync.dma_start(wr_sb[:], wr_d[:])
        bf_sb = wpool.tile([CELL, 1], f32, tag="bf")
        nc.sync.dma_start(bf_sb[:], bf_d[:])
        bi_sb = wpool.tile([CELL, 1], f32, tag="bi")
        nc.sync.dma_start(bi_sb[:], bi_d[:])
        dw_sb = wpool.tile([CELL, OUT_DIMS], f16, tag="dw")
        nc.sync.dma_start(dw_sb[:], dw_d[:])
        # dense bias broadcast across all 128 partitions
        db_sb = wpool.tile([128, OUT_DIMS], f32, tag="db")
        db_ap = db_d[:]
        db_bcast = bass.AP(
            tensor=db_ap.tensor, offset=db_ap.offset, ap=[[0, 128]] + list(db_ap.ap)
        )
        nc.sync.dma_start(db_sb[:], db_bcast)

        # ---- state (two half-batch chains) ----
        cs = [spool.tile([CELL, HB], f16, tag=f"c{h}", name=f"c{h}") for h in range(2)]
        for c in cs:
            nc.vector.memset(c[:], 0.0)

        wkf = [wk_sb[0:64, 0:CELL], wk_sb[64:128, 0:CELL]]
        wki = [wk_sb[0:64, CELL : 2 * CELL], wk_sb[64:128, CELL : 2 * CELL]]
        wrf = wr_sb[:, 0:CELL]
        wri = wr_sb[:, CELL : 2 * CELL]

        # ---- recurrence ----
        for tp in range(ntp):
            xt = xpool.tile([128, BC], f16, tag="xt")
            nc.sync.dma_start(xt[:], xp_d[tp])
            for j in range(2):
                xj = xt[j * IN_DIMS : (j + 1) * IN_DIMS, :]
                zf = [zpool.tile([CELL, HB], f32, tag=f"zf{h}", name=f"zf{h}_{tp}_{j}") for h in range(2)]
                zi = [zpool.tile([CELL, HB], f32, tag=f"zi{h}", name=f"zi{h}_{tp}_{j}") for h in range(2)]
                # input projections (no dependency on state; prefetchable)
                for h in range(2):
                    xh = xj[:, h * HB : (h + 1) * HB]
                    nc.tensor.matmul(zf[h][:], wkf[j], xh, start=True, stop=False)
                for h in range(2):
                    xh = xj[:, h * HB : (h + 1) * HB]
                    nc.tensor.matmul(zi[h][:], wki[j], xh, start=True, stop=False)
                # recurrent matmuls + gate math per chain
                for h in range(2):
                    nc.tensor.matmul(zf[h][:], wrf, cs[h][:], start=False, stop=True)
                    nc.tensor.matmul(zi[h][:], wri, cs[h][:], start=False, stop=True)
                    f = gpool.tile([CELL, HB], f16, tag=f"f{h}")
                    nc.scalar.activation(f[:], zf[h][:], AF.Sigmoid, bias=bf_sb[:])
                    g = gpool.tile([CELL, HB], f16, tag=f"g{h}")
                    nc.scalar.activation(g[:], zi[h][:], AF.Tanh, bias=bi_sb[:])
                    d = gpool.tile([CELL, HB], f16, tag=f"d{h}")
                    nc.vector.tensor_tensor(d[:], cs[h][:], g[:], OP.subtract)
                    m = gpool.tile([CELL, HB], f16, tag=f"m{h}")
                    nc.vector.tensor_tensor(m[:], f[:], d[:], OP.mult)
                    nc.vector.tensor_tensor(cs[h][:], m[:], g[:], OP.add)

        # ---- dense head: out[b,:] = c_T[:,b] . dw + db ----
        for h in range(2):
            for k in range(HB // 128):
                ops = zpool.tile([128, OUT_DIMS], f32, tag="zf0")
                nc.tensor.matmul(
                    ops[:], cs[h][:, k * 128 : (k + 1) * 128], dw_sb[:],
                    start=True, stop=True,
                )
                osb = gpool.tile([128, OUT_DIMS], f32, tag="osb")
                nc.vector.tensor_tensor(osb[:], ops[:], db_sb[:], OP.add)
                b0 = h * HB + k * 128
                nc.sync.dma_start(out_d[b0 : b0 + 128, :], osb[:])

    nc.compile()
    return nc


def prep_host(inputs, kernel, recurrent_kernel, recurrent_bias, dense_w, dense_b,
              T=T_FULL):
    """Build per-core input maps (host-side layout transform + fp16 casts)."""
    x = np.asarray(inputs)
    B = x.shape[0]
    bc = B // NCORES
    wk1 = np.asarray(kernel, np.float16)
    wk = np.ascontiguousarray(np.concatenate([wk1, wk1], axis=0))
    wr = np.ascontiguousarray(np.asarray(recurrent_kernel, np.float16))
    rb = np.asarray(recurrent_bias, np.float32)
    bf = np.ascontiguousarray(rb[:CELL].reshape(CELL, 1))
    bi = np.ascontiguousarray(rb[CELL:].reshape(CELL, 1))
    dw = np.ascontiguousarray(np.asarray(dense_w, np.float16))
    db = np.ascontiguousarray(np.asarray(dense_b, np.float32))

    x16 = x.astype(np.float16)  # [B, T, 64]
    in_maps = []
    for c in range(NCORES):
        xc = x16[c * bc : (c + 1) * bc]  # [bc, T, 64]
        xt = np.ascontiguousarray(xc.transpose(1, 2, 0))  # [T, 64, bc]
        xp = xt.reshape(T // 2, 128, bc)  # t-pair packing: p = j*64+i
        in_maps.append(
            {"xp": xp, "wk": wk, "wr": wr, "bf": bf, "bi": bi, "dw": dw, "db": db}
        )
    return in_maps


_NC_CACHE = {}


def kernel(**inp):
    from concourse import bass_utils

    T = inp["inputs"].shape[1]
    if T not in _NC_CACHE:
        _NC_CACHE[T] = build_nc(T)
    nc = _NC_CACHE[T]
    in_maps = prep_host(**inp)
    res = bass_utils.run_bass_kernel_spmd(nc, in_maps, core_ids=list(range(NCORES)))
    out = np.concatenate([r["out"] for r in res.results], axis=0)
    return out.astype(np.float32)
